# revision 17
# baseline (speedup 1.0000x reference)
"""Trainium2 Bass kernel for external-key attention with additive bias.

Reference computation (b=8, n=1024, dim=448, heads=7, d=64):
    qv = x @ w_qv ; q, v = split(qv)
    dots = (einsum('bhnd,hmd->bhnm', q, ext_k) + ext_bias) * d**-0.5
    out  = softmax(dots) @ v  -> (b,n,448) @ w_out + b_out

Sharding: 1-D over query positions n. Core c owns query rows
r in [c*128, (c+1)*128) for ALL batches and heads; ext_bias (the
dominant HBM tensor) splits perfectly. Each core computes the
V-projection for its own rows (= its share of key positions); an
AllGather distributes full V (fp8).

v3 schedule:
  - dummy 4-byte collective issued first (absorbs the runtime's
    first-collective global barrier while input DMAs run)
  - V-proj -> fp8 AllGather in background
  - Q-proj with col-group-paired matmuls (rh0 -> psum[0:64],
    rh1 -> psum[64:128] run concurrently on separate column groups)
  - scores: per 512-col block the (b,r) columns are ordered
    (rh, b, rl) so both the bias-injection and the k.q matmul split
    into K=64 halves on row groups 0/64 -> concurrent pairs, halving
    PE time; exp on ScalarE (the bottleneck engine, kept pure-exp)
  - attn@V per head in fp8 with a 3D-AP rhs (one 128-col stream per
    (m-chunk, batch) stationary); softmax normalization via DVE
    reciprocal + GpSimd partition_broadcast + DVE multiply
  - output projection: wave A (batches 0-3) accumulates per head to
    shrink the tail; wave B after the attention loop
"""

import sys

sys.path.insert(0, "/opt/trn_rl_repo")

import numpy as np

HEADS = 7
D = 64
N = 1024
DIM = 448
B = 8
NCORES = 8
R = N // NCORES          # 128 query rows per core
BR = B * R               # 1024 row-columns per core
E = D + 1                # v columns + ones column = 65
KC = 4                   # contraction chunks for dim=448
KP = DIM // KC           # 112
SCALE = float(D) ** -0.5
TE = 64 * E              # columns of one head's V tile in SBUF = 4160
HB = B * E               # per-head shard cols in the AG buffers = 520

_CACHE = {}


def _np_bf16():
    from concourse import mybir
    return mybir.dt.np(mybir.dt.bfloat16)


def _np_f8():
    from concourse import mybir
    return mybir.dt.np(mybir.dt.float8e4)


def build_nc():
    """Build the SPMD Bass graph (same graph on all 8 cores)."""
    import concourse.bass as bass
    import concourse.bacc as bacc
    import concourse.tile as tile
    from concourse import mybir

    bf = mybir.dt.bfloat16
    f32 = mybir.dt.float32
    f8 = mybir.dt.float8e4

    nc = bacc.Bacc("TRN2", target_bir_lowering=False, debug=False,
                   num_devices=NCORES)

    # ---- per-core DRAM inputs (host-prepared layouts) ----
    xT_d = nc.dram_tensor("xT", [DIM, BR], bf, kind="ExternalInput")
    wqv_d = nc.dram_tensor("wqv", [DIM, 2 * DIM], bf, kind="ExternalInput")
    # kT duplicated vertically: rows 0:64 and 64:128 both hold k^T
    kT2_d = nc.dram_tensor("kT2", [2 * D, HEADS * N], bf, kind="ExternalInput")
    bias_d = nc.dram_tensor("bias", [R, HEADS * N], bf, kind="ExternalInput")
    # I64 tiled 8x horizontally, twice vertically
    irep_d = nc.dram_tensor("irep", [R, 512], bf, kind="ExternalInput")
    wout_d = nc.dram_tensor("wout", [D, HEADS * DIM], bf, kind="ExternalInput")
    bout_d = nc.dram_tensor("bout", [1, DIM], bf, kind="ExternalInput")
    out_d = nc.dram_tensor("out", [BR, DIM], f32, kind="ExternalOutput")

    # internal DRAM for the V all-gather (fp8): rows = the core's 128 m
    # positions, cols = (h, b, e)
    vsh_d = nc.dram_tensor("vsh", [R, HEADS * HB], bf)
    vfull_d = nc.dram_tensor("vfull", [NCORES * R, HEADS * HB], bf,
                             addr_space="Shared")

    with tile.TileContext(nc) as tc:
        with (
            tc.tile_pool(name="persist", bufs=1) as pp,
            tc.tile_pool(name="pT", bufs=HEADS) as ppT,
            tc.tile_pool(name="big", bufs=3) as pbig,
            tc.tile_pool(name="outsb", bufs=2) as pout,
            tc.tile_pool(name="norm", bufs=2) as pnorm,
            tc.tile_pool(name="nrec", bufs=1) as pnrec,
        ):
            # ---- persistent SBUF ----
            xT_sb = pbig.tile([KP, KC * BR], bf, tag="big")
            wqv_sb = pbig.tile([KP, KC * 2 * DIM], bf, tag="big")
            kT2_sb = pp.tile([2 * D, HEADS * N], bf, tag="kT2")
            bias_sb = pp.tile([R, HEADS * N], bf, tag="bias")
            irep_sb = pp.tile([R, 512], bf, tag="irep")
            wout_sb = pp.tile([D, HEADS * DIM], bf, tag="wout")
            bout_sb = pp.tile([1, DIM], bf, tag="bout")
            ones1 = pp.tile([1, R], bf, tag="ones1")
            qT2_sb = pp.tile([2 * D, HEADS * 512], bf, tag="qT2")
            vsh_sb = pp.tile([R, HEADS * HB], bf, tag="vsh")
            normout = pp.tile([D, HEADS * BR], bf, tag="normout")

            # ---- input DMAs (spread across queues; V-proj needs first) ----
            nc.sync.dma_start(
                out=wqv_sb[:].rearrange("p (c n) -> p c n", c=KC)
                [:, :, DIM:2 * DIM],
                in_=wqv_d.ap().rearrange("(c p) n -> p c n", p=KP)
                [:, :, DIM:2 * DIM])
            for kc in range(KC):
                nc.sync.dma_start(
                    out=xT_sb[:, kc * BR:(kc + 1) * BR],
                    in_=xT_d.ap()[kc * KP:(kc + 1) * KP, :])
            nc.scalar.dma_start(
                out=wqv_sb[:].rearrange("p (c n) -> p c n", c=KC)
                [:, :, 0:DIM],
                in_=wqv_d.ap().rearrange("(c p) n -> p c n", p=KP)
                [:, :, 0:DIM])
            # head 0's bias/kT/irep next on scalar: they gate the first
            # score matmuls; bulk bias/kT2 split across all three rings
            nc.scalar.dma_start(out=irep_sb[:], in_=irep_d.ap())
            nc.scalar.dma_start(out=bias_sb[:, 0:N], in_=bias_d.ap()[:, 0:N])
            nc.scalar.dma_start(out=kT2_sb[:, 0:N], in_=kT2_d.ap()[:, 0:N])
            nc.scalar.dma_start(out=bias_sb[:, N:3 * N],
                                in_=bias_d.ap()[:, N:3 * N])
            nc.scalar.dma_start(out=kT2_sb[:, N:3 * N],
                                in_=kT2_d.ap()[:, N:3 * N])
            nc.sync.dma_start(out=bias_sb[:, 3 * N:5 * N],
                              in_=bias_d.ap()[:, 3 * N:5 * N])
            nc.sync.dma_start(out=kT2_sb[:, 3 * N:5 * N],
                              in_=kT2_d.ap()[:, 3 * N:5 * N])
            nc.gpsimd.dma_start(out=bias_sb[:, 5 * N:HEADS * N],
                                in_=bias_d.ap()[:, 5 * N:HEADS * N])
            nc.gpsimd.dma_start(out=kT2_sb[:, 5 * N:HEADS * N],
                                in_=kT2_d.ap()[:, 5 * N:HEADS * N])
            nc.gpsimd.dma_start(out=wout_sb[:], in_=wout_d.ap())
            nc.gpsimd.dma_start(out=bout_sb[:], in_=bout_d.ap())
            nc.vector.memset(ones1[:], 1.0)

            # ---- phase 0: V projection for our rows, launch all-gather ----
            with tc.tile_pool(name="ps_early", bufs=2,
                              space="PSUM") as ps_e:
                for rb in range(B):
                    psv = ps_e.tile([128, 512], f32, tag="e")
                    for kc in range(KC):
                        nc.tensor.matmul(
                            psv[:, 0:DIM],
                            lhsT=xT_sb[:, kc * BR + rb * R:
                                       kc * BR + (rb + 1) * R],
                            rhs=wqv_sb[:, kc * 2 * DIM + DIM:
                                       (kc + 1) * 2 * DIM],
                            start=(kc == 0), stop=(kc == KC - 1))
                    nc.vector.tensor_copy(
                        vsh_sb[:].rearrange("p (h b e) -> p h b e",
                                            h=HEADS, b=B)[:, :, rb, 0:D],
                        psv[:, 0:DIM].rearrange("p (h e) -> p h e", h=HEADS))
                nc.vector.memset(
                    vsh_sb[:].rearrange("p (t e) -> p t e", e=E)
                    [:, :, D:E], 1.0)

                nc.sync.dma_start(out=vsh_d.ap(), in_=vsh_sb[:])
                nc.gpsimd.collective_compute(
                    "AllGather", mybir.AluOpType.bypass,
                    replica_groups=[list(range(NCORES))],
                    ins=[vsh_d.ap().opt()], outs=[vfull_d.ap().opt()])

                # ---- Q^T projection: col-group paired (rh0 | rh1) ----
                for h in range(HEADS):
                    psq = ps_e.tile([128, 512], f32, tag="e")
                    for kc in range(KC):
                        xr = xT_sb[:, kc * BR:(kc + 1) * BR].rearrange(
                            "p (b rh rl) -> p b rh rl", b=B, rh=2)
                        wq = wqv_sb[:, kc * 2 * DIM + h * D:
                                    kc * 2 * DIM + (h + 1) * D]
                        nc.tensor.matmul(
                            psq[0:D, :], lhsT=wq, rhs=xr[:, :, 0, :],
                            start=(kc == 0), stop=(kc == KC - 1))
                        nc.tensor.matmul(
                            psq[D:2 * D, :], lhsT=wq, rhs=xr[:, :, 1, :],
                            start=(kc == 0), stop=(kc == KC - 1))
                    nc.vector.tensor_copy(
                        qT2_sb[:, h * 512:(h + 1) * 512], psq[:])

            # ---- phase 1: scores + exp for ALL heads ----
            # col order per m-chunk: (rh, b, rl); bias-inject and k.q both
            # split into K=64 row-group pairs running concurrently
            pT_tiles = []
            with tc.tile_pool(name="ps_scores", bufs=2,
                              space="PSUM") as ps_s:
                for h in range(HEADS):
                    pT_t = ppT.tile([128, B * N], bf, tag="pT")
                    pT_tiles.append(pT_t)
                    for mcp in range(4):          # pairs of m-chunks
                        ps = ps_s.tile([128, 2 * BR], f32, tag="s")
                        for sub in range(2):
                            mc = 2 * mcp + sub
                            base = sub * BR
                            msl = slice(h * N + mc * R, h * N + (mc + 1) * R)
                            nc.tensor.matmul(
                                ps[:, base:base + 512],
                                lhsT=bias_sb[0:D, msl],
                                rhs=irep_sb[0:D, :],
                                start=True, stop=False)
                            nc.tensor.matmul(
                                ps[:, base + 512:base + 1024],
                                lhsT=bias_sb[D:2 * D, msl],
                                rhs=irep_sb[D:2 * D, :],
                                start=True, stop=False)
                            nc.tensor.matmul(
                                ps[:, base:base + 512],
                                lhsT=kT2_sb[0:D, msl],
                                rhs=qT2_sb[0:D, h * 512:(h + 1) * 512],
                                start=False, stop=True)
                            nc.tensor.matmul(
                                ps[:, base + 512:base + 1024],
                                lhsT=kT2_sb[D:2 * D, msl],
                                rhs=qT2_sb[D:2 * D, h * 512:(h + 1) * 512],
                                start=False, stop=True)
                        nc.scalar.activation(
                            pT_t[:, mcp * 2 * BR:(mcp + 1) * 2 * BR],
                            ps[:], mybir.ActivationFunctionType.Exp,
                            scale=SCALE)

            # ---- phase 2: attn@V + normalize + interleaved out-proj ----
            with (
                tc.tile_pool(name="ps_att", bufs=4, space="PSUM") as ps_a,
                tc.tile_pool(name="ps_po", bufs=4, space="PSUM") as ps_po,
            ):
                po_ts = [ps_po.tile([128, 448], f32, tag="po",
                                    name=f"po_{b}") for b in range(4)]
                for h in range(HEADS):
                    vhp = pbig if h < 3 else ppT
                    vht_tag = "big" if h < 3 else "pT"
                    vh_t = vhp.tile([R, TE], bf, tag=vht_tag,
                                    name=f"vh_{h}")
                    vq = (nc.sync, nc.scalar, nc.gpsimd)[h % 3]
                    vq.dma_start(
                        out=vh_t[:].rearrange("p (j c) -> p j c", c=HB),
                        in_=vfull_d.ap()
                        .rearrange("(j p) c -> p j c", p=R)
                        [:, :, h * HB:(h + 1) * HB])
                    pT_r = pT_tiles[h][:].rearrange(
                        "p (mc rh b rl) -> p mc rh b rl", mc=B, rh=2, b=B)
                    atts = [ps_a.tile([E, 512], f32, tag="a",
                                      name=f"att_{h}_{g}")
                            for g in range(2)]
                    for b in range(B):
                        att = atts[b // 4]
                        csl = slice((b % 4) * R, (b % 4 + 1) * R)
                        for mc in range(B):
                            nc.tensor.matmul(
                                att[:, csl],
                                lhsT=vh_t[:, (mc * B + b) * E:
                                          (mc * B + b + 1) * E],
                                rhs=pT_r[:, mc, :, b, :],
                                start=(mc == 0), stop=(mc == B - 1))
                    for g in range(2):
                        att = atts[g]
                        rec = pnrec.tile([1, 512], bf, tag="nr")
                        with nc.allow_low_precision("softmax denom recip"):
                            nc.vector.reciprocal(rec[:], att[D:E, :])
                        rep = pnorm.tile([D, 512], bf, tag="np")
                        nc.gpsimd.partition_broadcast(rep[:], rec[:])
                        nc.vector.tensor_mul(
                            normout[:, h * BR + g * 512:
                                    h * BR + (g + 1) * 512],
                            att[0:D, :], rep[:])
                    # wave A out-proj for the PREVIOUS head: its normout is
                    # ready by now, so these never stall the PE queue
                    if h > 0:
                        for b in range(4):
                            nc.tensor.matmul(
                                po_ts[b][:],
                                lhsT=normout[:, (h - 1) * BR + b * R:
                                             (h - 1) * BR + (b + 1) * R],
                                rhs=wout_sb[:, (h - 1) * DIM:h * DIM],
                                start=(h == 1), stop=False)

                # wave A: last head's contribution, then per-batch epilogue
                # interleaved with wave B accumulations
                for b in range(4):
                    nc.tensor.matmul(
                        po_ts[b][:],
                        lhsT=normout[:, (HEADS - 1) * BR + b * R:
                                     (HEADS - 1) * BR + (b + 1) * R],
                        rhs=wout_sb[:, (HEADS - 1) * DIM:HEADS * DIM],
                        start=False, stop=False)
                po_b = []
                for b in range(4):
                    nc.tensor.matmul(
                        po_ts[b][:], lhsT=ones1[:, 0:128], rhs=bout_sb[:],
                        start=False, stop=True)
                    ot = pout.tile([R, DIM], f32, tag="o")
                    nc.vector.tensor_copy(ot[:], po_ts[b][:])
                    oq = (nc.sync, nc.scalar, nc.gpsimd)[b % 3]
                    oq.dma_start(
                        out=out_d.ap()[b * R:(b + 1) * R, :], in_=ot[:])
                    po = ps_po.tile([128, 448], f32, tag="po",
                                    name=f"po_{b + 4}")
                    po_b.append(po)
                    for h in range(HEADS):
                        nc.tensor.matmul(
                            po[:],
                            lhsT=normout[:, h * BR + (b + 4) * R:
                                         h * BR + (b + 5) * R],
                            rhs=wout_sb[:, h * DIM:(h + 1) * DIM],
                            start=(h == 0), stop=False)
                for b in range(4, B):
                    po = po_b[b - 4]
                    nc.tensor.matmul(
                        po[:], lhsT=ones1[:, 0:128], rhs=bout_sb[:],
                        start=False, stop=True)
                    ot = pout.tile([R, DIM], f32, tag="o")
                    nc.vector.tensor_copy(ot[:], po[:])
                    oq = (nc.sync, nc.scalar, nc.gpsimd)[b % 3]
                    oq.dma_start(
                        out=out_d.ap()[b * R:(b + 1) * R, :], in_=ot[:])

    nc.compile()
    return nc


def _prep_inputs(x, w_qv, ext_k, ext_bias, w_out, b_out):
    """Host-side sharding + layout transforms (device time unaffected)."""
    bf = _np_bf16()
    x = np.asarray(x, np.float32)
    xT_full = np.ascontiguousarray(x.transpose(2, 0, 1))        # [448, 8, 1024]
    kT = np.ascontiguousarray(
        np.asarray(ext_k, np.float32).transpose(2, 0, 1)).reshape(D, HEADS * N)
    kT2 = np.concatenate([kT, kT], axis=0)                      # [128, 7168]
    wqv = np.asarray(w_qv, np.float32)
    wout = np.ascontiguousarray(
        np.asarray(w_out, np.float32).reshape(HEADS, D, DIM)
        .transpose(1, 0, 2)).reshape(D, HEADS * DIM)
    bout = np.asarray(b_out, np.float32).reshape(1, DIM)
    irep = np.tile(np.eye(D, dtype=np.float32), (2, 8))         # [128, 512]

    kT2 = kT2.astype(bf)
    wqv_b = wqv.astype(bf)
    wout_b = wout.astype(bf)
    bout_b = bout.astype(bf)
    irep_b = irep.astype(bf)

    in_maps = []
    eb = np.asarray(ext_bias, np.float32)
    for c in range(NCORES):
        r0 = c * R
        xT_c = np.ascontiguousarray(
            xT_full[:, :, r0:r0 + R]).reshape(DIM, BR).astype(bf)
        bias_c = np.ascontiguousarray(
            eb[:, r0:r0 + R, :].transpose(1, 0, 2)).reshape(R, HEADS * N).astype(bf)
        in_maps.append({
            "xT": xT_c, "wqv": wqv_b, "kT2": kT2, "bias": bias_c,
            "irep": irep_b, "wout": wout_b, "bout": bout_b,
        })
    return in_maps


def _get_nc():
    if "nc" not in _CACHE:
        _CACHE["nc"] = build_nc()
    return _CACHE["nc"]


def _install_ntff_shim():
    """Provide antenv.axon_hooks (missing on this image) so
    run_bass_kernel_spmd(trace=True) can capture NTFF profiles, and
    neuter the artifact upload (no bucket in this container)."""
    import types, contextlib, ctypes

    if "antenv.axon_hooks" not in sys.modules:
        so_path = "/opt/axon/libaxon_pjrt.so"
        lib = ctypes.CDLL(so_path)
        hook = None
        if hasattr(lib, "axon_start_nrt_profile"):
            lib.axon_start_nrt_profile.argtypes = [
                ctypes.POINTER(ctypes.c_int64), ctypes.c_size_t]
            lib.axon_start_nrt_profile.restype = ctypes.c_int64
            lib.axon_stop_nrt_profile.argtypes = [ctypes.c_char_p]
            lib.axon_stop_nrt_profile.restype = ctypes.c_int64

            @contextlib.contextmanager
            def hook(output_dir, device_ids):
                import jax
                jax.devices()
                if device_ids:
                    ids = (ctypes.c_int64 * len(device_ids))(*device_ids)
                    rc = lib.axon_start_nrt_profile(ids, len(device_ids))
                else:
                    rc = lib.axon_start_nrt_profile(None, 0)
                if rc != 0:
                    raise RuntimeError(f"axon_start_nrt_profile rc={rc}")
                try:
                    yield
                finally:
                    n = lib.axon_stop_nrt_profile(str(output_dir).encode())
                    print(f"ntff profile: {n} file(s) -> {output_dir}")

        mod = types.ModuleType("antenv.axon_hooks")
        mod.get_axon_ntff_profile_hook = lambda: hook
        mod.set_axon_ntff_profile_hook = lambda h: None
        sys.modules["antenv.axon_hooks"] = mod
        import antenv
        antenv.axon_hooks = mod

    import concourse.bass_utils as bu
    if not getattr(bu, "_upload_patched", False):
        bu.upload_artifacts = lambda tmpdir: tmpdir
        bu._upload_patched = True


def run(inputs, trace=False):
    """Run on hardware; returns (full_output, BassKernelResults)."""
    from concourse.bass_utils import run_bass_kernel_spmd
    if trace:
        _install_ntff_shim()
    nc = _get_nc()
    in_maps = _prep_inputs(**inputs)
    res = run_bass_kernel_spmd(nc, in_maps, core_ids=list(range(NCORES)),
                               trace=trace)
    out = np.zeros((B, N, DIM), np.float32)
    for c in range(NCORES):
        o = np.asarray(res.results[c]["out"], np.float32)
        out[:, c * R:(c + 1) * R, :] = o.reshape(B, R, DIM)
    return out, res


def kernel(x, w_qv, ext_k, ext_bias, w_out, b_out):
    out, _ = run(dict(x=x, w_qv=w_qv, ext_k=ext_k, ext_bias=ext_bias,
                      w_out=w_out, b_out=b_out))
    return out


if __name__ == "__main__":
    nc = _get_nc()
    print("built + compiled OK")


# revision 18
# speedup vs baseline: 1.3613x; 1.3613x over previous
"""Trainium2 Bass kernel for external-key attention with additive bias.

Reference computation (b=8, n=1024, dim=448, heads=7, d=64):
    qv = x @ w_qv ; q, v = split(qv)
    dots = (einsum('bhnd,hmd->bhnm', q, ext_k) + ext_bias) * d**-0.5
    out  = softmax(dots) @ v  -> (b,n,448) @ w_out + b_out

Sharding: 1-D over query positions n. Core c owns query rows
r in [c*128, (c+1)*128) for ALL batches and heads; ext_bias (the
dominant HBM tensor) splits perfectly. Each core computes the
V-projection for its own rows (= its share of key positions); an
AllGather distributes full V (fp8).

v3 schedule:
  - dummy 4-byte collective issued first (absorbs the runtime's
    first-collective global barrier while input DMAs run)
  - V-proj -> fp8 AllGather in background
  - Q-proj with col-group-paired matmuls (rh0 -> psum[0:64],
    rh1 -> psum[64:128] run concurrently on separate column groups)
  - scores: per 512-col block the (b,r) columns are ordered
    (rh, b, rl) so both the bias-injection and the k.q matmul split
    into K=64 halves on row groups 0/64 -> concurrent pairs, halving
    PE time; exp on ScalarE (the bottleneck engine, kept pure-exp)
  - attn@V per head in fp8 with a 3D-AP rhs (one 128-col stream per
    (m-chunk, batch) stationary); softmax normalization via DVE
    reciprocal + GpSimd partition_broadcast + DVE multiply
  - output projection: wave A (batches 0-3) accumulates per head to
    shrink the tail; wave B after the attention loop
"""

import sys

sys.path.insert(0, "/opt/trn_rl_repo")

import numpy as np

HEADS = 7
D = 64
N = 1024
DIM = 448
B = 8
NCORES = 8
R = N // NCORES          # 128 query rows per core
BR = B * R               # 1024 row-columns per core
E = D + 1                # v columns + ones column = 65
KC = 4                   # contraction chunks for dim=448
KP = DIM // KC           # 112
SCALE = float(D) ** -0.5
TE = 64 * E              # columns of one head's V tile in SBUF = 4160
HB = B * E               # per-head shard cols in the AG buffers = 520

_CACHE = {}


def _np_bf16():
    from concourse import mybir
    return mybir.dt.np(mybir.dt.bfloat16)


def _np_f8():
    from concourse import mybir
    return mybir.dt.np(mybir.dt.float8e4)


def build_nc():
    """Build the SPMD Bass graph (same graph on all 8 cores)."""
    import concourse.bass as bass
    import concourse.bacc as bacc
    import concourse.tile as tile
    from concourse import mybir

    bf = mybir.dt.bfloat16
    f32 = mybir.dt.float32
    f8 = mybir.dt.float8e4

    nc = bacc.Bacc("TRN2", target_bir_lowering=False, debug=False,
                   num_devices=NCORES)

    # ---- per-core DRAM inputs (host-prepared layouts) ----
    xT_d = nc.dram_tensor("xT", [DIM, BR], bf, kind="ExternalInput")
    wqv_d = nc.dram_tensor("wqv", [DIM, 2 * DIM], bf, kind="ExternalInput")
    # kT duplicated vertically: rows 0:64 and 64:128 both hold k^T
    kT2_d = nc.dram_tensor("kT2", [2 * D, HEADS * N], bf, kind="ExternalInput")
    bias_d = nc.dram_tensor("bias", [R, HEADS * N], bf, kind="ExternalInput")
    # I64 tiled 8x horizontally, twice vertically
    irep_d = nc.dram_tensor("irep", [R, 512], bf, kind="ExternalInput")
    wout_d = nc.dram_tensor("wout", [D, HEADS * DIM], bf, kind="ExternalInput")
    bout_d = nc.dram_tensor("bout", [1, DIM], bf, kind="ExternalInput")
    out_d = nc.dram_tensor("out", [BR, DIM], f32, kind="ExternalOutput")

    # internal DRAM for the V all-gather (fp8): rows = the core's 128 m
    # positions, cols = (h, b, e)
    vsh_d = nc.dram_tensor("vsh", [R, HEADS * HB], bf)
    vfull_d = nc.dram_tensor("vfull", [NCORES * R, HEADS * HB], bf,
                             addr_space="Shared")

    with tile.TileContext(nc) as tc:
        with (
            tc.tile_pool(name="persist", bufs=1) as pp,
            tc.tile_pool(name="pT", bufs=HEADS) as ppT,
            tc.tile_pool(name="big", bufs=3) as pbig,
            tc.tile_pool(name="outsb", bufs=2) as pout,
            tc.tile_pool(name="norm", bufs=2) as pnorm,
            tc.tile_pool(name="nrec", bufs=1) as pnrec,
        ):
            # ---- persistent SBUF ----
            xT_sb = pbig.tile([KP, KC * BR], bf, tag="big")
            wqv_sb = pbig.tile([KP, KC * 2 * DIM], bf, tag="big")
            kT2_sb = pp.tile([2 * D, HEADS * N], bf, tag="kT2")
            bias_sb = pp.tile([R, HEADS * N], bf, tag="bias")
            irep_sb = pp.tile([R, 512], bf, tag="irep")
            wout_sb = pp.tile([D, HEADS * DIM], bf, tag="wout")
            bout_sb = pp.tile([1, DIM], bf, tag="bout")
            ones1 = pp.tile([1, R], bf, tag="ones1")
            qT2_sb = pp.tile([2 * D, HEADS * 512], bf, tag="qT2")
            vsh_sb = pp.tile([R, HEADS * HB], bf, tag="vsh")
            normout = pp.tile([D, HEADS * BR], bf, tag="normout")

            # ---- input DMAs (spread across queues; V-proj needs first) ----
            nc.sync.dma_start(
                out=wqv_sb[:].rearrange("p (c n) -> p c n", c=KC)
                [:, :, DIM:2 * DIM],
                in_=wqv_d.ap().rearrange("(c p) n -> p c n", p=KP)
                [:, :, DIM:2 * DIM])
            for kc in range(KC):
                nc.sync.dma_start(
                    out=xT_sb[:, kc * BR:(kc + 1) * BR],
                    in_=xT_d.ap()[kc * KP:(kc + 1) * KP, :])
            nc.scalar.dma_start(
                out=wqv_sb[:].rearrange("p (c n) -> p c n", c=KC)
                [:, :, 0:DIM],
                in_=wqv_d.ap().rearrange("(c p) n -> p c n", p=KP)
                [:, :, 0:DIM])
            # head 0's bias/kT/irep next on scalar: they gate the first
            # score matmuls; bulk bias/kT2 split across all three rings
            nc.scalar.dma_start(out=irep_sb[:], in_=irep_d.ap())
            nc.scalar.dma_start(out=bias_sb[:, 0:N], in_=bias_d.ap()[:, 0:N])
            nc.scalar.dma_start(out=kT2_sb[:, 0:N], in_=kT2_d.ap()[:, 0:N])
            nc.scalar.dma_start(out=bias_sb[:, N:3 * N],
                                in_=bias_d.ap()[:, N:3 * N])
            nc.scalar.dma_start(out=kT2_sb[:, N:3 * N],
                                in_=kT2_d.ap()[:, N:3 * N])
            nc.sync.dma_start(out=bias_sb[:, 3 * N:5 * N],
                              in_=bias_d.ap()[:, 3 * N:5 * N])
            nc.sync.dma_start(out=kT2_sb[:, 3 * N:5 * N],
                              in_=kT2_d.ap()[:, 3 * N:5 * N])
            nc.gpsimd.dma_start(out=bias_sb[:, 5 * N:HEADS * N],
                                in_=bias_d.ap()[:, 5 * N:HEADS * N])
            nc.gpsimd.dma_start(out=kT2_sb[:, 5 * N:HEADS * N],
                                in_=kT2_d.ap()[:, 5 * N:HEADS * N])
            nc.gpsimd.dma_start(out=wout_sb[:], in_=wout_d.ap())
            nc.gpsimd.dma_start(out=bout_sb[:], in_=bout_d.ap())
            nc.vector.memset(ones1[:], 1.0)

            # ---- phase 0: V projection for our rows, launch all-gather ----
            with tc.tile_pool(name="ps_early", bufs=2,
                              space="PSUM") as ps_e:
                for rb in range(B):
                    psv = ps_e.tile([128, 512], f32, tag="e")
                    for kc in range(KC):
                        nc.tensor.matmul(
                            psv[:, 0:DIM],
                            lhsT=xT_sb[:, kc * BR + rb * R:
                                       kc * BR + (rb + 1) * R],
                            rhs=wqv_sb[:, kc * 2 * DIM + DIM:
                                       (kc + 1) * 2 * DIM],
                            start=(kc == 0), stop=(kc == KC - 1))
                    nc.vector.tensor_copy(
                        vsh_sb[:].rearrange("p (h b e) -> p h b e",
                                            h=HEADS, b=B)[:, :, rb, 0:D],
                        psv[:, 0:DIM].rearrange("p (h e) -> p h e", h=HEADS))
                nc.vector.memset(
                    vsh_sb[:].rearrange("p (t e) -> p t e", e=E)
                    [:, :, D:E], 1.0)

                nc.sync.dma_start(out=vsh_d.ap(), in_=vsh_sb[:])
                nc.gpsimd.collective_compute(
                    "AllGather", mybir.AluOpType.bypass,
                    replica_groups=[list(range(NCORES))],
                    ins=[vsh_d.ap().opt()], outs=[vfull_d.ap().opt()])

                # ---- Q^T projection: col-group paired (rh0 | rh1) ----
                for h in range(HEADS):
                    psq = ps_e.tile([128, 512], f32, tag="e")
                    for kc in range(KC):
                        xr = xT_sb[:, kc * BR:(kc + 1) * BR].rearrange(
                            "p (b rh rl) -> p b rh rl", b=B, rh=2)
                        wq = wqv_sb[:, kc * 2 * DIM + h * D:
                                    kc * 2 * DIM + (h + 1) * D]
                        nc.tensor.matmul(
                            psq[0:D, :], lhsT=wq, rhs=xr[:, :, 0, :],
                            start=(kc == 0), stop=(kc == KC - 1))
                        nc.tensor.matmul(
                            psq[D:2 * D, :], lhsT=wq, rhs=xr[:, :, 1, :],
                            start=(kc == 0), stop=(kc == KC - 1))
                    nc.vector.tensor_copy(
                        qT2_sb[:, h * 512:(h + 1) * 512], psq[:])

            # ---- phase 1: scores + exp for ALL heads ----
            # col order per m-chunk: (rh, b, rl); bias-inject and k.q both
            # split into K=64 row-group pairs running concurrently
            pT_tiles = []
            with tc.tile_pool(name="ps_scores", bufs=2,
                              space="PSUM") as ps_s:
                for h in range(HEADS):
                    pT_t = ppT.tile([128, B * N], bf, tag="pT")
                    pT_tiles.append(pT_t)
                    for mcp in range(4):          # pairs of m-chunks
                        ps = ps_s.tile([128, 2 * BR], f32, tag="s")
                        for sub in range(2):
                            mc = 2 * mcp + sub
                            base = sub * BR
                            msl = slice(h * N + mc * R, h * N + (mc + 1) * R)
                            nc.tensor.matmul(
                                ps[:, base:base + 512],
                                lhsT=bias_sb[0:D, msl],
                                rhs=irep_sb[0:D, :],
                                start=True, stop=False)
                            nc.tensor.matmul(
                                ps[:, base + 512:base + 1024],
                                lhsT=bias_sb[D:2 * D, msl],
                                rhs=irep_sb[D:2 * D, :],
                                start=True, stop=False)
                            nc.tensor.matmul(
                                ps[:, base:base + 512],
                                lhsT=kT2_sb[0:D, msl],
                                rhs=qT2_sb[0:D, h * 512:(h + 1) * 512],
                                start=False, stop=True)
                            nc.tensor.matmul(
                                ps[:, base + 512:base + 1024],
                                lhsT=kT2_sb[D:2 * D, msl],
                                rhs=qT2_sb[D:2 * D, h * 512:(h + 1) * 512],
                                start=False, stop=True)
                        nc.scalar.activation(
                            pT_t[:, mcp * 2 * BR:(mcp + 1) * 2 * BR],
                            ps[:], mybir.ActivationFunctionType.Exp,
                            scale=SCALE)

            # ---- phase 2: attn@V + normalize + interleaved out-proj ----
            with (
                tc.tile_pool(name="ps_att", bufs=4, space="PSUM") as ps_a,
                tc.tile_pool(name="ps_po", bufs=4, space="PSUM") as ps_po,
            ):
                po_ts = [ps_po.tile([128, 448], f32, tag="po",
                                    name=f"po_{b}") for b in range(4)]
                for h in range(HEADS):
                    vhp = pbig if h < 3 else ppT
                    vht_tag = "big" if h < 3 else "pT"
                    vh_t = vhp.tile([R, TE], bf, tag=vht_tag,
                                    name=f"vh_{h}")
                    nc.sync.dma_start(
                        out=vh_t[:].rearrange("p (j c) -> p j c", c=HB),
                        in_=vfull_d.ap()
                        .rearrange("(j p) c -> p j c", p=R)
                        [:, :, h * HB:(h + 1) * HB])
                    pT_r = pT_tiles[h][:].rearrange(
                        "p (mc rh b rl) -> p mc rh b rl", mc=B, rh=2, b=B)
                    atts = [ps_a.tile([E, 512], f32, tag="a",
                                      name=f"att_{h}_{g}")
                            for g in range(2)]
                    for b in range(B):
                        att = atts[b // 4]
                        csl = slice((b % 4) * R, (b % 4 + 1) * R)
                        for mc in range(B):
                            nc.tensor.matmul(
                                att[:, csl],
                                lhsT=vh_t[:, (mc * B + b) * E:
                                          (mc * B + b + 1) * E],
                                rhs=pT_r[:, mc, :, b, :],
                                start=(mc == 0), stop=(mc == B - 1))
                    for g in range(2):
                        att = atts[g]
                        rec = pnrec.tile([1, 512], bf, tag="nr")
                        with nc.allow_low_precision("softmax denom recip"):
                            nc.vector.reciprocal(rec[:], att[D:E, :])
                        rep = pnorm.tile([D, 512], bf, tag="np")
                        nc.gpsimd.partition_broadcast(rep[:], rec[:])
                        nc.vector.tensor_mul(
                            normout[:, h * BR + g * 512:
                                    h * BR + (g + 1) * 512],
                            att[0:D, :], rep[:])
                    # wave A out-proj for the PREVIOUS head: its normout is
                    # ready by now, so these never stall the PE queue
                    if h > 0:
                        for b in range(4):
                            nc.tensor.matmul(
                                po_ts[b][:],
                                lhsT=normout[:, (h - 1) * BR + b * R:
                                             (h - 1) * BR + (b + 1) * R],
                                rhs=wout_sb[:, (h - 1) * DIM:h * DIM],
                                start=(h == 1), stop=False)

                # wave A: last head's contribution, then per-batch epilogue
                # interleaved with wave B accumulations
                for b in range(4):
                    nc.tensor.matmul(
                        po_ts[b][:],
                        lhsT=normout[:, (HEADS - 1) * BR + b * R:
                                     (HEADS - 1) * BR + (b + 1) * R],
                        rhs=wout_sb[:, (HEADS - 1) * DIM:HEADS * DIM],
                        start=False, stop=False)
                po_b = []
                for b in range(4):
                    nc.tensor.matmul(
                        po_ts[b][:], lhsT=ones1[:, 0:128], rhs=bout_sb[:],
                        start=False, stop=True)
                    ot = pout.tile([R, DIM], f32, tag="o")
                    nc.vector.tensor_copy(ot[:], po_ts[b][:])
                    oq = (nc.sync, nc.scalar, nc.gpsimd)[b % 3]
                    oq.dma_start(
                        out=out_d.ap()[b * R:(b + 1) * R, :], in_=ot[:])
                    po = ps_po.tile([128, 448], f32, tag="po",
                                    name=f"po_{b + 4}")
                    po_b.append(po)
                    for h in range(HEADS):
                        nc.tensor.matmul(
                            po[:],
                            lhsT=normout[:, h * BR + (b + 4) * R:
                                         h * BR + (b + 5) * R],
                            rhs=wout_sb[:, h * DIM:(h + 1) * DIM],
                            start=(h == 0), stop=False)
                for b in range(4, B):
                    po = po_b[b - 4]
                    nc.tensor.matmul(
                        po[:], lhsT=ones1[:, 0:128], rhs=bout_sb[:],
                        start=False, stop=True)
                    ot = pout.tile([R, DIM], f32, tag="o")
                    nc.vector.tensor_copy(ot[:], po[:])
                    oq = (nc.sync, nc.scalar, nc.gpsimd)[b % 3]
                    oq.dma_start(
                        out=out_d.ap()[b * R:(b + 1) * R, :], in_=ot[:])

    nc.compile()
    return nc


def _prep_inputs(x, w_qv, ext_k, ext_bias, w_out, b_out):
    """Host-side sharding + layout transforms (device time unaffected)."""
    bf = _np_bf16()
    x = np.asarray(x, np.float32)
    xT_full = np.ascontiguousarray(x.transpose(2, 0, 1))        # [448, 8, 1024]
    kT = np.ascontiguousarray(
        np.asarray(ext_k, np.float32).transpose(2, 0, 1)).reshape(D, HEADS * N)
    kT2 = np.concatenate([kT, kT], axis=0)                      # [128, 7168]
    wqv = np.asarray(w_qv, np.float32)
    wout = np.ascontiguousarray(
        np.asarray(w_out, np.float32).reshape(HEADS, D, DIM)
        .transpose(1, 0, 2)).reshape(D, HEADS * DIM)
    bout = np.asarray(b_out, np.float32).reshape(1, DIM)
    irep = np.tile(np.eye(D, dtype=np.float32), (2, 8))         # [128, 512]

    kT2 = kT2.astype(bf)
    wqv_b = wqv.astype(bf)
    wout_b = wout.astype(bf)
    bout_b = bout.astype(bf)
    irep_b = irep.astype(bf)

    in_maps = []
    eb = np.asarray(ext_bias, np.float32)
    for c in range(NCORES):
        r0 = c * R
        xT_c = np.ascontiguousarray(
            xT_full[:, :, r0:r0 + R]).reshape(DIM, BR).astype(bf)
        bias_c = np.ascontiguousarray(
            eb[:, r0:r0 + R, :].transpose(1, 0, 2)).reshape(R, HEADS * N).astype(bf)
        in_maps.append({
            "xT": xT_c, "wqv": wqv_b, "kT2": kT2, "bias": bias_c,
            "irep": irep_b, "wout": wout_b, "bout": bout_b,
        })
    return in_maps


def _get_nc():
    if "nc" not in _CACHE:
        _CACHE["nc"] = build_nc()
    return _CACHE["nc"]


def _install_ntff_shim():
    """Provide antenv.axon_hooks (missing on this image) so
    run_bass_kernel_spmd(trace=True) can capture NTFF profiles, and
    neuter the artifact upload (no bucket in this container)."""
    import types, contextlib, ctypes

    if "antenv.axon_hooks" not in sys.modules:
        so_path = "/opt/axon/libaxon_pjrt.so"
        lib = ctypes.CDLL(so_path)
        hook = None
        if hasattr(lib, "axon_start_nrt_profile"):
            lib.axon_start_nrt_profile.argtypes = [
                ctypes.POINTER(ctypes.c_int64), ctypes.c_size_t]
            lib.axon_start_nrt_profile.restype = ctypes.c_int64
            lib.axon_stop_nrt_profile.argtypes = [ctypes.c_char_p]
            lib.axon_stop_nrt_profile.restype = ctypes.c_int64

            @contextlib.contextmanager
            def hook(output_dir, device_ids):
                import jax
                jax.devices()
                if device_ids:
                    ids = (ctypes.c_int64 * len(device_ids))(*device_ids)
                    rc = lib.axon_start_nrt_profile(ids, len(device_ids))
                else:
                    rc = lib.axon_start_nrt_profile(None, 0)
                if rc != 0:
                    raise RuntimeError(f"axon_start_nrt_profile rc={rc}")
                try:
                    yield
                finally:
                    n = lib.axon_stop_nrt_profile(str(output_dir).encode())
                    print(f"ntff profile: {n} file(s) -> {output_dir}")

        mod = types.ModuleType("antenv.axon_hooks")
        mod.get_axon_ntff_profile_hook = lambda: hook
        mod.set_axon_ntff_profile_hook = lambda h: None
        sys.modules["antenv.axon_hooks"] = mod
        import antenv
        antenv.axon_hooks = mod

    import concourse.bass_utils as bu
    if not getattr(bu, "_upload_patched", False):
        bu.upload_artifacts = lambda tmpdir: tmpdir
        bu._upload_patched = True


def run(inputs, trace=False):
    """Run on hardware; returns (full_output, BassKernelResults)."""
    from concourse.bass_utils import run_bass_kernel_spmd
    if trace:
        _install_ntff_shim()
    nc = _get_nc()
    in_maps = _prep_inputs(**inputs)
    res = run_bass_kernel_spmd(nc, in_maps, core_ids=list(range(NCORES)),
                               trace=trace)
    out = np.zeros((B, N, DIM), np.float32)
    for c in range(NCORES):
        o = np.asarray(res.results[c]["out"], np.float32)
        out[:, c * R:(c + 1) * R, :] = o.reshape(B, R, DIM)
    return out, res


def kernel(x, w_qv, ext_k, ext_bias, w_out, b_out):
    out, _ = run(dict(x=x, w_qv=w_qv, ext_k=ext_k, ext_bias=ext_bias,
                      w_out=w_out, b_out=b_out))
    return out


if __name__ == "__main__":
    nc = _get_nc()
    print("built + compiled OK")
